# revision 6
# baseline (speedup 1.0000x reference)
"""Trainium2 Bass kernel for nn_MultiHeadAttention (B=2,S=2048,D=1024,H=16,DK=DV=64).

Sharding: 8 cores = 2 batch groups x 4 cores.
  core c: b = c//4, g = c%4, heads [4g, 4g+4), output-token quarter [512g, 512g+512).
Per core:
  - project Q/K/V for its 4 heads (Q^T/K^T in [dk, s] layout, V in [s, dv] layout)
  - attention per head: S = Q K^T (+ mask*NEG), E = exp(S/8) (+row sums),
    P = E/l -> softmax_attn output; E^T via PE transpose; qkv^T = V^T-aug @ E^T
    (extra ones-column gives row sums again); qkv^T normalized by 1/l.
  - partial fc: y_part = qkv^T.T @ W_fc_slice^T (contract local 256 dims),
    ReduceScatter(add) over the 4-core group -> this core's 512-token slice,
    + residual, LayerNorm -> y_out.
Host assembles the full outputs from the disjoint per-core slices.
"""

import numpy as np

import concourse.bass as bass
import concourse.tile as tile
from concourse import bacc, mybir
from concourse.bass import ts
from concourse.bass_utils import run_bass_kernel_spmd
from concourse.masks import make_identity

F32 = mybir.dt.float32
F32R = mybir.dt.float32r
U8 = mybir.dt.uint8

B, S, D, H, DK, DV = 2, 2048, 1024, 16, 64, 64
P = 128
HPC = 4                 # heads per core
LDV = HPC * DV          # 256 local value dims
SC = S // P             # 16 token chunks
DC = D // P             # 8 d chunks
NQS = 4                 # q supers
QSUB = 4                # q sub chunks per super
SCALE = 1.0 / np.sqrt(DK)
NEG = -1e9
EPS = 1e-5
SQ = S // 4             # 512, token quarter

# ---------------------------------------------------------------- kernel build

def _build():
    nc = bacc.Bacc("TRN2", target_bir_lowering=False, debug=False, num_devices=8)

    xq = nc.dram_tensor("xq", [S, D], F32, kind="ExternalInput")
    xk = nc.dram_tensor("xk", [S, D], F32, kind="ExternalInput")
    xv = nc.dram_tensor("xv", [S, D], F32, kind="ExternalInput")
    mask = nc.dram_tensor("mask", [S, S], U8, kind="ExternalInput")
    wq = nc.dram_tensor("wq", [LDV, D], F32, kind="ExternalInput")
    wk = nc.dram_tensor("wk", [LDV, D], F32, kind="ExternalInput")
    wv = nc.dram_tensor("wv", [LDV, D], F32, kind="ExternalInput")
    wfc = nc.dram_tensor("wfc", [D, LDV], F32, kind="ExternalInput")   # W_fc[:, local dv]
    xres = nc.dram_tensor("xres", [SQ, D], F32, kind="ExternalInput")  # residual rows
    gamma = nc.dram_tensor("gamma", [D], F32, kind="ExternalInput")
    beta = nc.dram_tensor("beta", [D], F32, kind="ExternalInput")

    p_out = nc.dram_tensor("p_out", [HPC, S, S], F32, kind="ExternalOutput")
    y_out = nc.dram_tensor("y_out", [SQ, D], F32, kind="ExternalOutput")

    x_tiled = {"q": xq.rearrange("(t p) d -> p t d", p=P),
               "k": xk.rearrange("(t p) d -> p t d", p=P),
               "v": xv.rearrange("(t p) d -> p t d", p=P)}
    mask_tiled = mask.rearrange("(t p) k -> p t k", p=P)

    with tile.TileContext(nc) as tc:
        import contextlib
        with contextlib.ExitStack() as ctx:
            singles = ctx.enter_context(tc.tile_pool(name="singles", bufs=1))
            ident = singles.tile([P, P], F32)
            make_identity(nc, ident)

            # persistent products
            QT = singles.tile([P, 2, S], F32R, name="QT")    # [pair-row, pair, s]
            KT = singles.tile([P, 2, S], F32R, name="KT")
            VA = singles.tile([P, SC, HPC, DV + 1], F32R, name="VA")
            WfcT = singles.tile([P, 2, D], F32R, name="WfcT")
            qkvT = singles.tile([P, 2, S], F32R, name="qkvT")
            # memset cannot write f32r; stage ones in f32 and cast-copy
            onesf = singles.tile([1, DV], F32, name="onesf")
            nc.vector.memset(onesf, 1.0)
            ones1 = singles.tile([1, DV], F32R, name="ones1")
            nc.vector.tensor_copy(out=ones1, in_=onesf)
            onesc = singles.tile([P, SC * HPC], F32, name="onesc")
            nc.vector.memset(onesc, 1.0)
            nc.vector.tensor_copy(
                out=VA[:, :, :, DV],
                in_=onesc.rearrange("p (a b) -> p a b", a=SC))

            # ---------------- phase A: weight + input transposes, projections
            with tc.tile_pool(name="phA", bufs=2) as phA, \
                 tc.tile_pool(name="phA1", bufs=1) as phA1, \
                 tc.tile_pool(name="psA", bufs=2, space="PSUM") as psA:

                # W^T for q/k/v: [256, 1024] -> [128, 8, 256] f32r
                WT = {}
                for nm, w in (("q", wq), ("k", wk), ("v", wv)):
                    w_sb = phA.tile([P, 2, D], F32, name="w_sb", tag="w_sb")
                    nc.sync.dma_start(out=w_sb, in_=w.rearrange("(c p) d -> p c d", p=P))
                    wt = phA1.tile([P, DC, LDV], F32R, name=f"w{nm}T")
                    for c in range(2):
                        for dc in range(DC):
                            pw = psA.tile([P, P], F32, tag="trA")
                            nc.tensor.transpose(pw, w_sb[:, c, ts(dc, P)], ident)
                            nc.scalar.copy(out=wt[:, dc, ts(c, P)], in_=pw)
                    WT[nm] = wt

                # WfcT: wfc [1024, 256] -> [128, 2, 1024] f32r
                wfc_sb = phA.tile([P, DC, LDV], F32, name="wfc_sb", tag="w_sb2")
                nc.sync.dma_start(out=wfc_sb, in_=wfc.rearrange("(c p) d -> p c d", p=P))
                for dc in range(DC):
                    for c in range(2):
                        pw = psA.tile([P, P], F32, tag="trA")
                        nc.tensor.transpose(pw, wfc_sb[:, dc, ts(c, P)], ident)
                        nc.scalar.copy(out=WfcT[:, c, ts(dc, P)], in_=pw)

                # X^T per input (streamed per token chunk), then projections
                for nm in ("q", "k", "v"):
                    XT = phA1.tile([P, DC, S], F32R, name="XT", tag="XT")
                    for t in range(SC):
                        x_sb = phA.tile([P, D], F32, name="x_sb", tag="x_sb")
                        nc.sync.dma_start(out=x_sb, in_=x_tiled[nm][:, t, :])
                        for half in range(2):
                            pt = psA.tile([P, 4 * P], F32, tag="trX")
                            for j in range(4):
                                nc.tensor.transpose(
                                    pt[:, ts(j, P)], x_sb[:, ts(half * 4 + j, P)], ident)
                            eng = nc.vector if (t + half) % 2 == 0 else nc.scalar
                            if eng is nc.vector:
                                nc.vector.tensor_copy(
                                    out=XT[:, half * 4:half * 4 + 4, ts(t, P)], in_=pt)
                            else:
                                nc.scalar.copy(
                                    out=XT[:, half * 4:half * 4 + 4, ts(t, P)], in_=pt)

                    if nm in ("q", "k"):
                        dst = QT if nm == "q" else KT
                        wt = WT[nm]
                        for pair in range(2):
                            for ns in range(4):
                                pp = psA.tile([P, 512], F32, tag="prj")
                                for dc in range(DC):
                                    nc.tensor.matmul(
                                        pp, wt[:, dc, ts(pair, P)],
                                        XT[:, dc, ts(ns, 512)],
                                        start=(dc == 0), stop=(dc == DC - 1))
                                eng = nc.vector if (pair + ns) % 2 == 0 else nc.scalar
                                if eng is nc.vector:
                                    nc.vector.tensor_copy(out=dst[:, pair, ts(ns, 512)], in_=pp)
                                else:
                                    nc.scalar.copy(out=dst[:, pair, ts(ns, 512)], in_=pp)
                    else:
                        wt = WT["v"]
                        for t in range(SC):
                            pp = psA.tile([P, LDV], F32, tag="prjv")
                            for dc in range(DC):
                                nc.tensor.matmul(
                                    pp, XT[:, dc, ts(t, P)], wt[:, dc, :],
                                    start=(dc == 0), stop=(dc == DC - 1))
                            # scatter [128, 4*64] -> VA[:, t, h, 0:64]
                            nc.vector.tensor_copy(
                                out=VA[:, t, :, 0:DV],
                                in_=pp.rearrange("p (h v) -> p h v", h=HPC))

            # ---------------- phase B: attention
            with tc.tile_pool(name="phB", bufs=2) as phB, \
                 tc.tile_pool(name="phB1", bufs=1) as phB1, \
                 tc.tile_pool(name="psS", bufs=1, space="PSUM") as psS, \
                 tc.tile_pool(name="psT", bufs=2, space="PSUM") as psT, \
                 tc.tile_pool(name="psQ", bufs=1, space="PSUM") as psQ:

                for qs in range(NQS):
                    # mask chunks for this q super, reused across heads
                    mnegs = []
                    for qc in range(QSUB):
                        m_u8 = phB.tile([P, S], U8, name="m_u8", tag="m_u8")
                        nc.sync.dma_start(out=m_u8, in_=mask_tiled[:, qs * QSUB + qc, :])
                        mneg = phB1.tile([P, S], F32, name=f"mneg{qc}", tag=f"mneg{qc}")
                        nc.vector.tensor_scalar_mul(mneg, m_u8, NEG)
                        mnegs.append(mneg)

                    for h in range(HPC):
                        hp, hr = h // 2, (h % 2) * DK
                        ET = phB1.tile([P, SC, 512], F32R, name="ET", tag="ET")
                        lrows = []
                        for qc in range(QSUB):
                            sp = psS.tile([P, S], F32, tag="sc")
                            for ns in range(4):
                                nc.tensor.matmul(
                                    sp[:, ts(ns, 512)],
                                    QT[hr:hr + DK, hp, ts(qs * QSUB + qc, P)],
                                    KT[hr:hr + DK, hp, ts(ns, 512)],
                                    start=True, stop=True)
                            nc.vector.tensor_tensor(
                                out=sp, in0=sp, in1=mnegs[qc], op=mybir.AluOpType.add)
                            e_sub = phB.tile([P, S], F32, name="e_sub", tag="e_sub")
                            lcol = phB.tile([P, 1], F32, name="lcol", tag="lcol")
                            nc.scalar.activation(
                                out=e_sub, in_=sp,
                                func=mybir.ActivationFunctionType.Exp,
                                scale=SCALE, accum_out=lcol)
                            rl = phB.tile([P, 1], F32, name="rl", tag="rl")
                            nc.vector.reciprocal(out=rl, in_=lcol)
                            p_sub = phB.tile([P, S], F32, name="p_sub", tag="p_sub")
                            nc.vector.tensor_scalar_mul(p_sub, e_sub, rl)
                            nc.sync.dma_start(
                                out=p_out[h, ts(qs * QSUB + qc, P), :], in_=p_sub)
                            # transpose E chunk -> ET[:, kc, qc*128:+128]
                            for kc4 in range(4):
                                pt = psT.tile([P, 4 * P], F32, tag="trE")
                                for j in range(4):
                                    nc.tensor.transpose(
                                        pt[:, ts(j, P)],
                                        e_sub[:, ts(kc4 * 4 + j, P)], ident)
                                if (qc + kc4) % 2 == 0:
                                    nc.vector.tensor_copy(
                                        out=ET[:, kc4 * 4:kc4 * 4 + 4, ts(qc, P)], in_=pt)
                                else:
                                    nc.scalar.copy(
                                        out=ET[:, kc4 * 4:kc4 * 4 + 4, ts(qc, P)], in_=pt)

                        # AV: qkv^T [65, 512] for this (h, qs)
                        qp = psQ.tile([DV + 1, 512], F32, tag="qk")
                        for kc in range(SC):
                            nc.tensor.matmul(
                                qp, VA[:, kc, h, :], ET[:, kc, :],
                                start=(kc == 0), stop=(kc == SC - 1))
                        # 1/l row, replicate to 64 partitions via ones-outer-product
                        lrow = phB.tile([1, 512], F32, name="lrow", tag="lrow")
                        nc.vector.tensor_copy(out=lrow, in_=qp[DV:DV + 1, :])
                        rlrow_f = phB.tile([1, 512], F32, name="rlrow_f", tag="rlrow_f")
                        nc.vector.reciprocal(out=rlrow_f, in_=lrow)
                        rlrow = phB.tile([1, 512], F32R, name="rlrow", tag="rlrow")
                        nc.vector.tensor_copy(out=rlrow, in_=rlrow_f)
                        rp = psQ.tile([DV, 512], F32, tag="rep")
                        nc.tensor.matmul(rp, ones1, rlrow, start=True, stop=True)
                        rl_rep = phB.tile([DV, 512], F32, name="rl_rep", tag="rl_rep")
                        nc.scalar.copy(out=rl_rep, in_=rp)
                        nc.vector.tensor_tensor(
                            out=qkvT[hr:hr + DV, hp, ts(qs, 512)],
                            in0=qp[0:DV, :], in1=rl_rep, op=mybir.AluOpType.mult)

            # ---------------- phase C: partial fc, reduce-scatter, LN
            with tc.tile_pool(name="phC", bufs=2) as phC, \
                 tc.tile_pool(name="phC1", bufs=1) as phC1, \
                 tc.tile_pool(name="psC", bufs=2, space="PSUM") as psC, \
                 tc.tile_pool(name="dramC", bufs=1, space="DRAM") as dramC:

                y_part = dramC.tile([SC, P, D], F32)
                rs_out = dramC.tile([QSUB, P, D], F32)

                for sc in range(SC):
                    yp = psC.tile([P, D], F32, tag="yp")
                    for half in range(2):
                        for dvc in range(2):
                            nc.tensor.matmul(
                                yp[:, ts(half, 512)],
                                qkvT[:, dvc, ts(sc, P)],
                                WfcT[:, dvc, ts(half, 512)],
                                start=(dvc == 0), stop=(dvc == 1))
                    y_sb = phC.tile([P, D], F32, name="y_sb", tag="y_sb")
                    nc.scalar.copy(out=y_sb, in_=yp)
                    nc.sync.dma_start(out=y_part[sc], in_=y_sb)

                nc.gpsimd.collective_compute(
                    "ReduceScatter",
                    mybir.AluOpType.add,
                    replica_groups=[[0, 1, 2, 3], [4, 5, 6, 7]],
                    ins=[y_part.opt()],
                    outs=[rs_out.opt()],
                )

                # LN over this core's 512-token slice
                gb = phC1.tile([P, 2, D], F32, name="gb")
                nc.sync.dma_start(
                    out=gb[:, 0, :],
                    in_=bass.AP(tensor=gamma.ap().tensor, offset=0,
                                ap=[[0, P], [1, D]]))
                nc.sync.dma_start(
                    out=gb[:, 1, :],
                    in_=bass.AP(tensor=beta.ap().tensor, offset=0,
                                ap=[[0, P], [1, D]]))
                eps_sb = phC1.tile([P, 1], F32, name="eps_sb")
                nc.vector.memset(eps_sb, EPS)
                xres_t = xres.rearrange("(t p) d -> p t d", p=P)

                for sc in range(QSUB):
                    y_sb = phC.tile([P, D], F32, name="yln", tag="yln")
                    nc.sync.dma_start(out=y_sb, in_=rs_out[sc])
                    res_sb = phC.tile([P, D], F32, name="res_sb", tag="res_sb")
                    nc.sync.dma_start(out=res_sb, in_=xres_t[:, sc, :])
                    nc.vector.tensor_tensor(
                        out=y_sb, in0=y_sb, in1=res_sb, op=mybir.AluOpType.add)
                    stats = phC.tile([P, 2, 6], F32, name="stats", tag="stats")
                    for g2 in range(2):
                        nc.vector.bn_stats(out=stats[:, g2, :], in_=y_sb[:, ts(g2, 512)])
                    mv = phC.tile([P, 2], F32, name="mv", tag="mv")
                    nc.vector.bn_aggr(out=mv, in_=stats)
                    rstd = phC.tile([P, 1], F32, name="rstd", tag="rstd")
                    nc.scalar.activation(
                        out=rstd, in_=mv[:, 1:2],
                        func=mybir.ActivationFunctionType.Sqrt, bias=eps_sb)
                    nc.vector.reciprocal(out=rstd, in_=rstd)
                    nc.vector.tensor_scalar(
                        out=y_sb, in0=y_sb, scalar1=mv[:, 0:1], scalar2=rstd,
                        op0=mybir.AluOpType.subtract, op1=mybir.AluOpType.mult)
                    nc.vector.tensor_tensor(
                        out=y_sb, in0=y_sb, in1=gb[:, 0, :], op=mybir.AluOpType.mult)
                    nc.vector.tensor_tensor(
                        out=y_sb, in0=y_sb, in1=gb[:, 1, :], op=mybir.AluOpType.add)
                    nc.sync.dma_start(
                        out=y_out.rearrange("(t p) d -> p t d", p=P)[:, sc, :],
                        in_=y_sb)

    nc.compile()
    return nc


_CACHE = {}


def _get_nc():
    if "nc" not in _CACHE:
        _CACHE["nc"] = _build()
    return _CACHE["nc"]


def make_in_maps(input_Q, input_K, input_V, attn_mask, W_Q, W_K, W_V, W_fc,
                 ln_gamma, ln_beta):
    mask_u8 = np.ascontiguousarray(attn_mask).astype(np.uint8)
    in_maps = []
    for c in range(8):
        b, g = c // 4, c % 4
        hs = slice(g * LDV, (g + 1) * LDV)
        in_maps.append({
            "xq": np.ascontiguousarray(input_Q[b]),
            "xk": np.ascontiguousarray(input_K[b]),
            "xv": np.ascontiguousarray(input_V[b]),
            "mask": np.ascontiguousarray(mask_u8[b]),
            "wq": np.ascontiguousarray(W_Q[hs]),
            "wk": np.ascontiguousarray(W_K[hs]),
            "wv": np.ascontiguousarray(W_V[hs]),
            "wfc": np.ascontiguousarray(W_fc[:, hs]),
            "xres": np.ascontiguousarray(input_Q[b, g * SQ:(g + 1) * SQ]),
            "gamma": np.ascontiguousarray(ln_gamma),
            "beta": np.ascontiguousarray(ln_beta),
        })
    return in_maps


def assemble(results):
    normed = np.empty((B, S, D), np.float32)
    softmax = np.empty((B, H, S, S), np.float32)
    for c in range(8):
        b, g = c // 4, c % 4
        softmax[b, g * HPC:(g + 1) * HPC] = results[c]["p_out"]
        normed[b, g * SQ:(g + 1) * SQ] = results[c]["y_out"]
    return normed, softmax


def kernel(**inputs):
    nc = _get_nc()
    in_maps = make_in_maps(**inputs)
    res = run_bass_kernel_spmd(nc, in_maps, core_ids=list(range(8)))
    return assemble(res.results)


# revision 11
# speedup vs baseline: 34602.0850x; 34602.0850x over previous
"""Trainium2 Bass kernel for nn_MultiHeadAttention (B=2,S=2048,D=1024,H=16,DK=DV=64).

Sharding: 8 cores = 2 batch groups x 4 cores.
  core c: b = c//4, g = c%4, heads [4g, 4g+4), output-token quarter [512g, 512g+512).
Per core:
  - project Q/K/V for its 4 heads (Q^T/K^T in [dk, s] layout, V in [s, dv] layout)
  - attention per head, two independent streams sharing the QK^T work:
      S-stream  (softmax output): S = Q K^T (+ NEG*mask via PE identity-matmul),
                E = exp(S/8) with ACT row-sum accumulation, P = E/l -> p_out.
      ST-stream (attention-value): S^T = K Q^T (+ NEG*mask^T), E^T = exp(S^T/8)
                written straight PSUM->SBUF by ACT; qkv^T = [V|1]^T @ E^T
                (ones column yields row sums), normalized by 1/l.
  - blocks that the mask fully masks / leaves clear are specialized away at
    build time (the build is cached keyed on the block-level mask pattern;
    per-block values still come from the runtime mask tensor).
  - partial fc: y_part = qkv^T.T @ W_fc_slice^T (contract local 256 dims),
    ReduceScatter(add) over the 4-core group -> this core's 512-token slice,
    + residual, LayerNorm -> y_out.
Host assembles the full outputs from the disjoint per-core slices.
Note: a fully-masked attention row would produce NaN here (reference gives a
uniform distribution); causal and all-clear masks never hit that case.
"""

import numpy as np

import concourse.bass as bass
import concourse.tile as tile
from concourse import bacc, mybir
from concourse.bass import ts
from concourse.bass_utils import run_bass_kernel_spmd
from concourse.masks import make_identity

F32 = mybir.dt.float32
F32R = mybir.dt.float32r
BF16 = mybir.dt.bfloat16
U8 = mybir.dt.uint8

B, S, D, H, DK, DV = 2, 2048, 1024, 16, 64, 64
P = 128
HPC = 4                 # heads per core
LDV = HPC * DV          # 256 local value dims
SC = S // P             # 16 token chunks
DC = D // P             # 8 d chunks
NQS = 4                 # q supers (512 tokens each)
QSUB = 4                # q sub chunks per super
NS = 4                  # 512-wide column blocks per row
SCALE = 1.0 / np.sqrt(DK)
NEG = -1e9
EPS = 1e-5
SQ = S // 4             # 512, token quarter

CLEAR, PARTIAL, MASKED = 0, 1, 2

# ---------------------------------------------------------------- kernel build

def _build(side_s=None, side_t=None, collective=True):
    """side_s[qc][ns]: state of S-block rows [qc*128,+128) x cols [ns*512,+512).
    side_t[kc][qs]: state of S^T-block rows [kc*128,+128) x cols [qs*512,+512).
    None means 'assume everything PARTIAL' (fully general)."""
    if side_s is None:
        side_s = [[PARTIAL] * NS for _ in range(SC)]
    if side_t is None:
        side_t = [[PARTIAL] * NQS for _ in range(SC)]
    any_mask = any(st != CLEAR for row in side_s for st in row)

    n_dev = 8 if collective else 1
    nc = bacc.Bacc("TRN2", target_bir_lowering=False, debug=False, num_devices=n_dev)

    xq = nc.dram_tensor("xq", [S, D], F32, kind="ExternalInput")
    xk = nc.dram_tensor("xk", [S, D], F32, kind="ExternalInput")
    xv = nc.dram_tensor("xv", [S, D], F32, kind="ExternalInput")
    mask = nc.dram_tensor("mask", [S, S], U8, kind="ExternalInput")
    wq = nc.dram_tensor("wq", [LDV, D], F32, kind="ExternalInput")
    wk = nc.dram_tensor("wk", [LDV, D], F32, kind="ExternalInput")
    wv = nc.dram_tensor("wv", [LDV, D], F32, kind="ExternalInput")
    wfc = nc.dram_tensor("wfc", [D, LDV], F32, kind="ExternalInput")   # W_fc[:, local dv]
    xres = nc.dram_tensor("xres", [SQ, D], F32, kind="ExternalInput")  # residual rows
    gamma = nc.dram_tensor("gamma", [D], F32, kind="ExternalInput")
    beta = nc.dram_tensor("beta", [D], F32, kind="ExternalInput")

    p_out = nc.dram_tensor("p_out", [HPC, S, S], F32, kind="ExternalOutput")
    y_out = nc.dram_tensor("y_out", [SQ, D], F32, kind="ExternalOutput")

    x_tiled = {"q": xq.rearrange("(t p) d -> p t d", p=P),
               "k": xk.rearrange("(t p) d -> p t d", p=P),
               "v": xv.rearrange("(t p) d -> p t d", p=P)}
    mask_tiled = mask.rearrange("(t p) k -> p t k", p=P)

    with tile.TileContext(nc) as tc:
        import contextlib
        with contextlib.ExitStack() as ctx:
            singles = ctx.enter_context(tc.tile_pool(name="singles", bufs=1))
            ident = singles.tile([P, P], F32)
            make_identity(nc, ident)
            identb = singles.tile([P, P], BF16, name="identb")
            nc.vector.tensor_copy(out=identb, in_=ident)

            # persistent products
            QT = singles.tile([P, 2, S], F32R, name="QT")    # [pair-row, pair, s]
            KT = singles.tile([P, 2, S], F32R, name="KT")
            VA = singles.tile([P, SC, HPC, DV + 1], F32R, name="VA")
            WfcT = singles.tile([P, 2, D], F32R, name="WfcT")
            qkvT = singles.tile([P, 2, S], F32R, name="qkvT")
            # memset cannot write f32r; stage ones in f32 and cast-copy
            onesf = singles.tile([1, DV], F32, name="onesf")
            nc.vector.memset(onesf, 1.0)
            ones1 = singles.tile([1, DV], F32R, name="ones1")
            nc.vector.tensor_copy(out=ones1, in_=onesf)
            onesc = singles.tile([P, SC * HPC], F32, name="onesc")
            nc.vector.memset(onesc, 1.0)
            nc.vector.tensor_copy(
                out=VA[:, :, :, DV],
                in_=onesc.rearrange("p (a b) -> p a b", a=SC))

            # ---------------- phase A: weight + input transposes, projections
            with tc.tile_pool(name="phA", bufs=2) as phA, \
                 tc.tile_pool(name="phA1", bufs=1) as phA1, \
                 tc.tile_pool(name="psA", bufs=2, space="PSUM") as psA:

                # W^T for q/k/v: [256, 1024] -> [128, 8, 256] f32r
                WT = {}
                for nm, w in (("q", wq), ("k", wk), ("v", wv)):
                    w_sb = phA.tile([P, 2, D], F32, name="w_sb", tag="w_sb")
                    nc.sync.dma_start(out=w_sb, in_=w.rearrange("(c p) d -> p c d", p=P))
                    wt = phA1.tile([P, DC, LDV], F32R, name=f"w{nm}T")
                    for c in range(2):
                        for dc in range(DC):
                            pw = psA.tile([P, P], F32, tag="trA")
                            nc.tensor.transpose(pw, w_sb[:, c, ts(dc, P)], ident)
                            nc.scalar.copy(out=wt[:, dc, ts(c, P)], in_=pw)
                    WT[nm] = wt

                # WfcT: wfc [1024, 256] -> [128, 2, 1024] f32r
                wfc_sb = phA.tile([P, DC, LDV], F32, name="wfc_sb", tag="w_sb2")
                nc.sync.dma_start(out=wfc_sb, in_=wfc.rearrange("(c p) d -> p c d", p=P))
                for dc in range(DC):
                    for c in range(2):
                        pw = psA.tile([P, P], F32, tag="trA")
                        nc.tensor.transpose(pw, wfc_sb[:, dc, ts(c, P)], ident)
                        nc.scalar.copy(out=WfcT[:, c, ts(dc, P)], in_=pw)

                # X^T per input (streamed per token chunk), then projections
                for nm in ("q", "k", "v"):
                    XT = phA1.tile([P, DC, S], F32R, name="XT", tag="XT")
                    for t in range(SC):
                        x_sb = phA.tile([P, D], F32, name="x_sb", tag="x_sb")
                        nc.sync.dma_start(out=x_sb, in_=x_tiled[nm][:, t, :])
                        for half in range(2):
                            pt = psA.tile([P, 4 * P], F32, tag="trX")
                            for j in range(4):
                                nc.tensor.transpose(
                                    pt[:, ts(j, P)], x_sb[:, ts(half * 4 + j, P)], ident)
                            if (t + half) % 2 == 0:
                                nc.vector.tensor_copy(
                                    out=XT[:, half * 4:half * 4 + 4, ts(t, P)], in_=pt)
                            else:
                                nc.scalar.copy(
                                    out=XT[:, half * 4:half * 4 + 4, ts(t, P)], in_=pt)

                    if nm in ("q", "k"):
                        dst = QT if nm == "q" else KT
                        wt = WT[nm]
                        for pair in range(2):
                            for ns in range(4):
                                pp = psA.tile([P, 512], F32, tag="prj")
                                for dc in range(DC):
                                    nc.tensor.matmul(
                                        pp, wt[:, dc, ts(pair, P)],
                                        XT[:, dc, ts(ns, 512)],
                                        start=(dc == 0), stop=(dc == DC - 1))
                                if (pair + ns) % 2 == 0:
                                    nc.vector.tensor_copy(out=dst[:, pair, ts(ns, 512)], in_=pp)
                                else:
                                    nc.scalar.copy(out=dst[:, pair, ts(ns, 512)], in_=pp)
                    else:
                        wt = WT["v"]
                        for t in range(SC):
                            pp = psA.tile([P, LDV], F32, tag="prjv")
                            for dc in range(DC):
                                nc.tensor.matmul(
                                    pp, XT[:, dc, ts(t, P)], wt[:, dc, :],
                                    start=(dc == 0), stop=(dc == DC - 1))
                            nc.vector.tensor_copy(
                                out=VA[:, t, :, 0:DV],
                                in_=pp.rearrange("p (h v) -> p h v", h=HPC))

            # ---------------- phase B: attention
            with tc.tile_pool(name="phB", bufs=2) as phB, \
                 tc.tile_pool(name="phB1", bufs=1) as phB1, \
                 tc.tile_pool(name="psS", bufs=1, space="PSUM") as psS, \
                 tc.tile_pool(name="psT", bufs=2, space="PSUM") as psT, \
                 tc.tile_pool(name="psQ", bufs=1, space="PSUM") as psQ:

                for qs in range(NQS):
                    # --- bf16 NEG*mask tiles for this q super
                    # S-side: one [128, 512] tile per (qc, partial ns)
                    mrow = {}
                    mneg_s = {}
                    for qc in range(QSUB):
                        gqc = qs * QSUB + qc
                        need_row = any(side_s[gqc][ns] != CLEAR for ns in range(NS))
                        need_row |= any(side_t[kc][qs] == PARTIAL for kc in range(SC))
                        if not need_row:
                            continue
                        m_u8 = phB.tile([P, S], U8, name="m_u8", tag=f"m_u8{qc}")
                        nc.sync.dma_start(out=m_u8, in_=mask_tiled[:, gqc, :])
                        mrow[qc] = m_u8
                        for ns in range(NS):
                            if side_s[gqc][ns] != CLEAR:
                                mn = phB1.tile([P, 512], BF16, name=f"mns{qc}_{ns}",
                                               tag=f"mns{qc}_{ns}")
                                nc.vector.tensor_scalar_mul(
                                    mn, m_u8[:, ts(ns, 512)], NEG)
                                mneg_s[(qc, ns)] = mn
                    # S^T-side: transposed [128k, 512q] tiles per partial kc
                    mneg_t = {}
                    for kc in range(SC):
                        if side_t[kc][qs] != PARTIAL:
                            continue
                        pt = psT.tile([P, 512], BF16, tag="st")
                        for qc in range(QSUB):
                            mb = phB.tile([P, P], BF16, name="mb", tag="mb")
                            nc.vector.tensor_scalar_mul(
                                mb, mrow[qc][:, ts(kc, P)], NEG)
                            nc.tensor.transpose(pt[:, ts(qc, P)], mb, identb)
                        mt = phB1.tile([P, 512], BF16, name=f"mnt{kc}", tag=f"mnt{kc % 8}")
                        nc.vector.tensor_copy(out=mt, in_=pt)
                        mneg_t[kc] = mt

                    for h in range(HPC):
                        hp, hr = h // 2, (h % 2) * DK
                        # ---- S-stream: P output rows for this (h, qs)
                        for qc in range(QSUB):
                            gqc = qs * QSUB + qc
                            sp = psS.tile([P, S], F32, tag="sc")
                            for ns in range(NS):
                                st = side_s[gqc][ns]
                                if st != MASKED:
                                    nc.tensor.matmul(
                                        sp[:, ts(ns, 512)],
                                        QT[hr:hr + DK, hp, ts(gqc, P)],
                                        KT[hr:hr + DK, hp, ts(ns, 512)],
                                        start=True, stop=(st == CLEAR))
                                if st != CLEAR:
                                    nc.tensor.matmul(
                                        sp[:, ts(ns, 512)], identb, mneg_s[(qc, ns)],
                                        start=(st == MASKED), stop=True)
                            e_sub = phB.tile([P, S], F32, name="e_sub", tag="e_sub")
                            lcol = phB.tile([P, 1], F32, name="lcol", tag="lcol")
                            nc.scalar.activation(
                                out=e_sub, in_=sp,
                                func=mybir.ActivationFunctionType.Exp,
                                scale=SCALE, accum_out=lcol)
                            rl = phB.tile([P, 1], F32, name="rl", tag="rl")
                            nc.vector.reciprocal(out=rl, in_=lcol)
                            p_sub = phB.tile([P, S], F32, name="p_sub", tag="p_sub")
                            nc.gpsimd.tensor_scalar_mul(p_sub, e_sub, rl)
                            nc.sync.dma_start(out=p_out[h, ts(gqc, P), :], in_=p_sub)

                        # ---- ST-stream: E^T for this (h, qs)
                        ET = phB1.tile([P, SC, 512], F32R, name="ET",
                                       tag="ET")
                        for kc in range(SC):
                            st = side_t[kc][qs]
                            if st == MASKED:
                                continue
                            tp2 = psT.tile([P, 512], F32, tag="st")
                            if st != MASKED:
                                nc.tensor.matmul(
                                    tp2,
                                    KT[hr:hr + DK, hp, ts(kc, P)],
                                    QT[hr:hr + DK, hp, ts(qs, 512)],
                                    start=True, stop=(st == CLEAR))
                            if st == PARTIAL:
                                nc.tensor.matmul(
                                    tp2, identb, mneg_t[kc],
                                    start=False, stop=True)
                            nc.scalar.activation(
                                out=ET[:, kc, :], in_=tp2,
                                func=mybir.ActivationFunctionType.Exp, scale=SCALE)

                        # ---- AV: qkv^T [65, 512] for this (h, qs)
                        live = [kc for kc in range(SC) if side_t[kc][qs] != MASKED]
                        qp = psQ.tile([DV + 1, 512], F32, tag="qk")
                        for i, kc in enumerate(live):
                            nc.tensor.matmul(
                                qp, VA[:, kc, h, :], ET[:, kc, :],
                                start=(i == 0), stop=(i == len(live) - 1))
                        # 1/l row, replicate to 64 partitions via ones-outer-product
                        lrow = phB.tile([1, 512], F32, name="lrow", tag="lrow")
                        nc.vector.tensor_copy(out=lrow, in_=qp[DV:DV + 1, :])
                        rlrow_f = phB.tile([1, 512], F32, name="rlrow_f", tag="rlrow_f")
                        nc.vector.reciprocal(out=rlrow_f, in_=lrow)
                        rlrow = phB.tile([1, 512], F32R, name="rlrow", tag="rlrow")
                        nc.vector.tensor_copy(out=rlrow, in_=rlrow_f)
                        rp = psQ.tile([DV, 512], F32, tag="rep")
                        nc.tensor.matmul(rp, ones1, rlrow, start=True, stop=True)
                        rl_rep = phB.tile([DV, 512], F32, name="rl_rep", tag="rl_rep")
                        nc.scalar.copy(out=rl_rep, in_=rp)
                        nc.vector.tensor_tensor(
                            out=qkvT[hr:hr + DV, hp, ts(qs, 512)],
                            in0=qp[0:DV, :], in1=rl_rep, op=mybir.AluOpType.mult)

            # ---------------- phase C: partial fc, reduce-scatter, LN
            with tc.tile_pool(name="phC", bufs=2) as phC, \
                 tc.tile_pool(name="phC1", bufs=1) as phC1, \
                 tc.tile_pool(name="psC", bufs=2, space="PSUM") as psC, \
                 tc.tile_pool(name="dramC", bufs=1, space="DRAM") as dramC:

                y_part = dramC.tile([SC, P, D], F32)
                rs_out = dramC.tile([QSUB, P, D], F32)

                for sc in range(SC):
                    yp = psC.tile([P, D], F32, tag="yp")
                    for half in range(2):
                        for dvc in range(2):
                            nc.tensor.matmul(
                                yp[:, ts(half, 512)],
                                qkvT[:, dvc, ts(sc, P)],
                                WfcT[:, dvc, ts(half, 512)],
                                start=(dvc == 0), stop=(dvc == 1))
                    y_sb = phC.tile([P, D], F32, name="y_sb", tag="y_sb")
                    nc.scalar.copy(out=y_sb, in_=yp)
                    nc.sync.dma_start(out=y_part[sc], in_=y_sb)

                if collective:
                    nc.gpsimd.collective_compute(
                        "ReduceScatter",
                        mybir.AluOpType.add,
                        replica_groups=[[0, 1, 2, 3], [4, 5, 6, 7]],
                        ins=[y_part.opt()],
                        outs=[rs_out.opt()],
                    )
                else:  # timing-sim build: stand-in for the collective
                    nc.sync.dma_start(out=rs_out[:], in_=y_part[0:QSUB])

                # LN over this core's 512-token slice
                gb = phC1.tile([P, 2, D], F32, name="gb")
                nc.sync.dma_start(
                    out=gb[:, 0, :],
                    in_=bass.AP(tensor=gamma.ap().tensor, offset=0,
                                ap=[[0, P], [1, D]]))
                nc.sync.dma_start(
                    out=gb[:, 1, :],
                    in_=bass.AP(tensor=beta.ap().tensor, offset=0,
                                ap=[[0, P], [1, D]]))
                eps_sb = phC1.tile([P, 1], F32, name="eps_sb")
                nc.vector.memset(eps_sb, EPS)
                xres_t = xres.rearrange("(t p) d -> p t d", p=P)

                for sc in range(QSUB):
                    y_sb = phC.tile([P, D], F32, name="yln", tag="yln")
                    nc.sync.dma_start(out=y_sb, in_=rs_out[sc])
                    res_sb = phC.tile([P, D], F32, name="res_sb", tag="res_sb")
                    nc.sync.dma_start(out=res_sb, in_=xres_t[:, sc, :])
                    nc.vector.tensor_tensor(
                        out=y_sb, in0=y_sb, in1=res_sb, op=mybir.AluOpType.add)
                    stats = phC.tile([P, 2, 6], F32, name="stats", tag="stats")
                    for g2 in range(2):
                        nc.vector.bn_stats(out=stats[:, g2, :], in_=y_sb[:, ts(g2, 512)])
                    mv = phC.tile([P, 2], F32, name="mv", tag="mv")
                    nc.vector.bn_aggr(out=mv, in_=stats)
                    rstd = phC.tile([P, 1], F32, name="rstd", tag="rstd")
                    nc.scalar.activation(
                        out=rstd, in_=mv[:, 1:2],
                        func=mybir.ActivationFunctionType.Sqrt, bias=eps_sb)
                    nc.vector.reciprocal(out=rstd, in_=rstd)
                    nc.vector.tensor_scalar(
                        out=y_sb, in0=y_sb, scalar1=mv[:, 0:1], scalar2=rstd,
                        op0=mybir.AluOpType.subtract, op1=mybir.AluOpType.mult)
                    nc.vector.tensor_tensor(
                        out=y_sb, in0=y_sb, in1=gb[:, 0, :], op=mybir.AluOpType.mult)
                    nc.vector.tensor_tensor(
                        out=y_sb, in0=y_sb, in1=gb[:, 1, :], op=mybir.AluOpType.add)
                    nc.sync.dma_start(
                        out=y_out.rearrange("(t p) d -> p t d", p=P)[:, sc, :],
                        in_=y_sb)

    nc.compile()
    return nc


# ------------------------------------------------------------- host-side logic

def _mask_states(mask_union):
    """Block-level tri-state maps from the union (over b) of the bool mask."""
    m = mask_union
    side_s = [[PARTIAL] * NS for _ in range(SC)]
    side_t = [[PARTIAL] * NQS for _ in range(SC)]
    for qc in range(SC):
        rows = m[qc * P:(qc + 1) * P]
        for ns in range(NS):
            blk = rows[:, ns * 512:(ns + 1) * 512]
            if not blk.any():
                side_s[qc][ns] = CLEAR
            elif blk.all():
                side_s[qc][ns] = MASKED
    for kc in range(SC):
        cols = m[:, kc * P:(kc + 1) * P]
        for qs in range(NQS):
            blk = cols[qs * 512:(qs + 1) * 512]
            if not blk.any():
                side_t[kc][qs] = CLEAR
            elif blk.all():
                side_t[kc][qs] = MASKED
    return side_s, side_t


_CACHE = {}


def _get_nc(side_s, side_t):
    key = (tuple(map(tuple, side_s)), tuple(map(tuple, side_t)))
    if key not in _CACHE:
        _CACHE[key] = _build(side_s, side_t)
    return _CACHE[key]


def make_in_maps(input_Q, input_K, input_V, attn_mask, W_Q, W_K, W_V, W_fc,
                 ln_gamma, ln_beta):
    mask_u8 = np.ascontiguousarray(attn_mask).astype(np.uint8)
    in_maps = []
    for c in range(8):
        b, g = c // 4, c % 4
        hs = slice(g * LDV, (g + 1) * LDV)
        in_maps.append({
            "xq": np.ascontiguousarray(input_Q[b]),
            "xk": np.ascontiguousarray(input_K[b]),
            "xv": np.ascontiguousarray(input_V[b]),
            "mask": np.ascontiguousarray(mask_u8[b]),
            "wq": np.ascontiguousarray(W_Q[hs]),
            "wk": np.ascontiguousarray(W_K[hs]),
            "wv": np.ascontiguousarray(W_V[hs]),
            "wfc": np.ascontiguousarray(W_fc[:, hs]),
            "xres": np.ascontiguousarray(input_Q[b, g * SQ:(g + 1) * SQ]),
            "gamma": np.ascontiguousarray(ln_gamma),
            "beta": np.ascontiguousarray(ln_beta),
        })
    return in_maps


def assemble(results):
    normed = np.empty((B, S, D), np.float32)
    softmax = np.empty((B, H, S, S), np.float32)
    for c in range(8):
        b, g = c // 4, c % 4
        softmax[b, g * HPC:(g + 1) * HPC] = results[c]["p_out"]
        normed[b, g * SQ:(g + 1) * SQ] = results[c]["y_out"]
    return normed, softmax


def kernel(**inputs):
    mask_union = np.asarray(inputs["attn_mask"]).astype(bool).any(axis=0)
    side_s, side_t = _mask_states(mask_union)
    nc = _get_nc(side_s, side_t)
    in_maps = make_in_maps(**inputs)
    res = run_bass_kernel_spmd(nc, in_maps, core_ids=list(range(8)))
    return assemble(res.results)
